# revision 1
# baseline (speedup 1.0000x reference)
"""Trainium2 Bass kernel for a 2-layer GRU (H=10) + linear head.

Strategy (pure data parallel, 8 cores):
  - Shard batch B=1024 -> 128 per core; replicate the tiny weights.
  - Per core, the T=2048 recurrence runs as one fused chain covering BOTH
    GRU layers (layer 1 lags layer 0 by one step, software-pipelined).
  - State per (slot, batch) = [h0(10); h1(10)] on SBUF partitions 0:20;
    x_t lives in its own [1, CHUNK*128] per-chunk tile.  Two matmuls per
    step produce every linear term: a K=1 matmul for the rank-1 x
    contributions (waits only on the x DMA) and a K=20 matmul for the
    recurrent + layer-1-input terms (waits only on DVE) -- matmuls have
    a one-sync-wait budget so their deps must not mix.
  - PSUM layout per step ([116, 128] tile, batch in free dim).  Engine
    partition accesses may not cross a 32-partition quadrant boundary
    unless they start on one, so each 20-row gate block starts at a
    quadrant base:
      rows  0:20   z-gate preacts  (layer0 | layer1)
      rows 32:52   r-gate preacts
      rows 64:84   h-side n preacts (W_hn @ h)
      rows 96:116  x-side n preacts (W_in @ x)
    n_arg scratch lives in a separate [20, 128] PSUM tile.
  - Per step: 1 matmul (PE), sigmoid + tanh (ACT, same table set),
    2 fused scalar_tensor_tensor + 2 tensor_tensor (DVE), 2 elementwise
    (GPSIMD/Pool).  All biases ride free on ACT bias / STT scalar slots.
  - h2 states accumulate in the same SBUF chunks; DMA'd out every CHUNK
    steps.  The final linear head (10 -> 1) runs on host (a degenerate
    M=1 matmul on PE otherwise).
"""

import numpy as np

H = 10
B = 1024
T = 2048
NCORES = 8
BL = B // NCORES  # 128 batch rows per core
CHUNK = 64        # time slots per SBUF state chunk

_PROGRAM_CACHE = {}


def _dims(t_steps):
    nstep = t_steps + 1          # macro-steps 0..t (layer1 lags by one)
    nslot = nstep + 1            # state slots
    nch = (nslot + CHUNK - 1) // CHUNK
    return nstep, nslot, nch


def _build_program(t_steps):
    from contextlib import ExitStack

    import concourse.bass as bass
    import concourse.mybir as mybir

    fp32 = mybir.dt.float32
    Alu = mybir.AluOpType
    Act = mybir.ActivationFunctionType

    nstep, nslot, nch = _dims(t_steps)

    nc = bass.Bass()

    x_d = nc.declare_dram_parameter("xt", [nch, CHUNK * BL], fp32,
                                    isOutput=False)
    wrec_d = nc.declare_dram_parameter("w_rec", [2 * H + 1, 116], fp32,
                                       isOutput=False)  # row 20 = w_x
    bvec_d = nc.declare_dram_parameter("bvec", [116, 1], fp32,
                                       isOutput=False)
    h2_d = nc.declare_dram_parameter("h2", [nch, H, CHUNK * BL], fp32,
                                     isOutput=True)

    ctx = ExitStack()
    sb = lambda shape, name: ctx.enter_context(
        nc.sbuf_tensor(name, shape, fp32))
    ps = lambda shape, name: ctx.enter_context(
        nc.psum_tensor(name, shape, fp32))
    sem = lambda name: ctx.enter_context(nc.semaphore(name))

    wh_raw = sb([2 * H, 116], "wh_raw")
    wx_raw = sb([1, 116], "wx_raw")
    bv_raw = sb([116, 1], "bv_raw")
    w_h = sb([2 * H, 116], "w_h")
    w_x = sb([1, 116], "w_x")
    bv = sb([116, 1], "bv")
    b_hn2 = sb([52, 1], "b_hn2")   # b_hh n-gate at rows 32:52 (matches r)
    b_in2 = sb([20, 1], "b_in2")   # b_ih n-gate at rows 0:20 (matches tt)
    srz = sb([52, BL], "srz")
    tt = sb([20, BL], "tt")
    sn = sb([20, BL], "sn")
    omz = sb([20, BL], "omz")
    uu = sb([20, BL], "uu")
    vv = sb([20, BL], "vv")
    state = [sb([2 * H, CHUNK * BL], f"state{i}") for i in range(3)]
    xsb = [sb([1, CHUNK * BL], f"xsb{i}") for i in range(2)]
    P = [ps([116, BL], "P0"), ps([116, BL], "P1")]
    narg = ps([20, BL], "narg")

    sem_d = sem("sem_d")    # DVE: 1 prologue + 2/step (STT2, h')
    sem_a = sem("sem_a")    # ACT: 2/step
    sem_p = sem("sem_p")    # PE: 1/pair
    sem_g = sem("sem_g")    # Pool: 1/step
    dma_w = sem("dma_w")    # weight/bias DMAs
    dma_x = [sem("dma_xa"), sem("dma_xb")]      # x chunks, by parity
    dma_o = [sem(f"dma_o{i}") for i in range(3)]  # h2 out, by c%3

    def slot_ap(s, rows=slice(0, 2 * H)):
        return state[(s // CHUNK) % 3][rows, (s % CHUNK) * BL:
                                       (s % CHUNK) * BL + BL]

    def x_ap(s):
        return xsb[(s // CHUNK) % 2][:, (s % CHUNK) * BL:
                                     (s % CHUNK) * BL + BL]

    # sem_d value after h'(k) = 2k + 3;  after STT2(k) = 2k + 2
    def d_after_hp(k):
        return 2 * k + 3

    with nc.Block() as block:

        @block.sync
        def _(sp):
            sp.dma_start(wh_raw[:, :], wrec_d[0:2 * H]).then_inc(dma_w, 16)
            sp.dma_start(wx_raw[:, :], wrec_d[2 * H:2 * H + 1]).then_inc(
                dma_w, 16)
            sp.dma_start(bv_raw[:, :], bvec_d[:]).then_inc(dma_w, 16)
            sp.dma_start(xsb[0][:, :], x_d[0]).then_inc(dma_x[0], 16)
            if nch > 1:
                sp.dma_start(xsb[1][:, :], x_d[1]).then_inc(dma_x[1], 16)
            for c in range(nch):
                # stream out chunk c once its last h' lands
                last_k = min(64 * c + CHUNK - 2, nstep - 1)
                sp.wait_ge(sem_d, d_after_hp(last_k))
                sp.dma_start(h2_d[c], state[c % 3][H:2 * H, :]).then_inc(
                    dma_o[c % 3], 16)
                # refill x buffer (c+2) once pairs of chunk c are done
                if c + 2 < nch:
                    sp.wait_ge(sem_p, CHUNK * c + CHUNK)
                    sp.dma_start(xsb[c % 2][:, :], x_d[c + 2]).then_inc(
                        dma_x[c % 2], 16)

        @block.tensor
        def _(pe):
            for j in range(nstep):
                pe.wait_ge(sem_d, 1 if j == 0 else 2 * j + 1)
                nc.tensor.matmul(P[j % 2][:, :], w_h[:, :], slot_ap(j),
                                 start=True, stop=False)
                if j % CHUNK == 0:
                    c = j // CHUNK
                    pe.wait_ge(dma_x[c % 2], 16 * (c // 2 + 1))
                nc.tensor.matmul(P[j % 2][:, :], w_x[:, :], x_ap(j),
                                 start=False, stop=True).then_inc(sem_p)

        @block.scalar
        def _(act):
            for k in range(nstep):
                act.wait_ge(sem_p, k + 1)
                nc.scalar.activation(srz[:, :], P[k % 2][0:52, :],
                                     Act.Sigmoid,
                                     bias=bv[0:52, :]).then_inc(sem_a)
                act.wait_ge(sem_d, 2 * k + 2)
                nc.scalar.activation(sn[:, :], narg[:, :],
                                     Act.Tanh).then_inc(sem_a)

        @block.vector
        def _(dve):
            dve.wait_ge(dma_w, 48)
            nc.vector.tensor_copy(w_h[:, :], wh_raw[:, :])
            nc.vector.tensor_copy(w_x[:, :], wx_raw[:, :])
            nc.vector.tensor_copy(bv[:, :], bv_raw[:, :])
            nc.vector.tensor_copy(b_hn2[32:52, :], bv_raw[64:84, :])
            nc.vector.tensor_copy(b_in2[:, :], bv_raw[96:116, :])
            # h(0) = 0 for both slots 0 and 1 (k=0 writes only h0 rows)
            nc.vector.memset(state[0][0:2 * H, 0:2 * BL], 0.0).then_inc(
                sem_d)
            for k in range(nstep):
                dve.wait_ge(sem_a, 2 * k + 1)
                # t = (hn + b_hn) * r      (r @ rows 32:52)
                nc.vector.scalar_tensor_tensor(
                    tt[:, :], P[k % 2][64:84, :], b_hn2[32:52, :],
                    srz[32:52, :], op0=Alu.add, op1=Alu.mult)
                # n_arg = (xn + b_in) + t
                nc.vector.scalar_tensor_tensor(
                    narg[:, :], P[k % 2][96:116, :], b_in2[:, :],
                    tt[:, :], op0=Alu.add, op1=Alu.add).then_inc(sem_d)
                dve.wait_ge(sem_a, 2 * k + 2)
                dve.wait_ge(sem_g, k + 1)
                # h' = n*(1-z) + h*z
                nc.vector.tensor_mul(vv[:, :], sn[:, :], omz[:, :])
                if (k + 1) % CHUNK == 0 and (k + 1) // CHUNK >= 3:
                    # state tile reuse: wait for h2 DMA 3 chunks back
                    cp = (k + 1) // CHUNK
                    dve.wait_ge(dma_o[cp % 3], 16 * (cp // 3))
                if k == 0:
                    nc.vector.tensor_add(slot_ap(1, slice(0, H)),
                                         vv[0:H, :],
                                         uu[0:H, :]).then_inc(sem_d)
                else:
                    nc.vector.tensor_add(slot_ap(k + 1), vv[:, :],
                                         uu[:, :]).then_inc(sem_d)

        @block.gpsimd
        def _(gp):
            for k in range(nstep):
                gp.wait_ge(sem_a, 2 * k + 1)
                # omz = 1 - z ; u = h * z   (z @ rows 0:20)
                nc.gpsimd.tensor_scalar(omz[:, :], srz[0:20, :], -1.0, 1.0,
                                        op0=Alu.mult, op1=Alu.add)
                nc.gpsimd.tensor_mul(uu[:, :], slot_ap(k),
                                     srz[0:20, :]).then_inc(sem_g)

    return nc, nch


def _get_program(t_steps):
    if t_steps not in _PROGRAM_CACHE:
        _PROGRAM_CACHE[t_steps] = _build_program(t_steps)
    return _PROGRAM_CACHE[t_steps]


def _pack_weights(w_ih0, w_hh0, b_ih0, b_hh0, w_ih1, w_hh1, b_ih1, b_hh1):
    """Column layout (80): [r0 r1 z0 z1 hn0 hn1 xn0 xn1] x 10 each."""
    # column blocks: z0@0 z1@10 r0@32 r1@42 hn0@64 hn1@74 xn0@96 xn1@106
    w_rec = np.zeros((2 * H + 1, 116), np.float32)
    # rows 0:10 contract with h0
    w_rec[0:10, 32:42] = w_hh0[0:10, :].T      # r0
    w_rec[0:10, 0:10] = w_hh0[10:20, :].T      # z0
    w_rec[0:10, 64:74] = w_hh0[20:30, :].T     # hn0
    w_rec[0:10, 42:52] = w_ih1[0:10, :].T      # r1 (layer1 input side)
    w_rec[0:10, 10:20] = w_ih1[10:20, :].T     # z1
    w_rec[0:10, 106:116] = w_ih1[20:30, :].T   # xn1
    # rows 10:20 contract with h1
    w_rec[10:20, 42:52] = w_hh1[0:10, :].T     # r1
    w_rec[10:20, 10:20] = w_hh1[10:20, :].T    # z1
    w_rec[10:20, 74:84] = w_hh1[20:30, :].T    # hn1
    # row 20 contracts with x_t (rank-1 layer0 input side)
    w_rec[20, 32:42] = w_ih0[0:10, 0]          # r0
    w_rec[20, 0:10] = w_ih0[10:20, 0]          # z0
    w_rec[20, 96:106] = w_ih0[20:30, 0]        # xn0

    bvec = np.zeros((116, 1), np.float32)
    bvec[32:42, 0] = b_ih0[0:10] + b_hh0[0:10]     # r0
    bvec[42:52, 0] = b_ih1[0:10] + b_hh1[0:10]     # r1
    bvec[0:10, 0] = b_ih0[10:20] + b_hh0[10:20]    # z0
    bvec[10:20, 0] = b_ih1[10:20] + b_hh1[10:20]   # z1
    bvec[64:74, 0] = b_hh0[20:30]                  # hn0 (inside r*)
    bvec[74:84, 0] = b_hh1[20:30]                  # hn1
    bvec[96:106, 0] = b_ih0[20:30]                 # xn0
    bvec[106:116, 0] = b_ih1[20:30]                # xn1
    return w_rec, bvec


def run(x, w_ih0, w_hh0, b_ih0, b_hh0, w_ih1, w_hh1, b_ih1, b_hh1,
        w_lin, b_lin, t_steps=T, trace=False):
    from concourse.bass_utils import run_bass_kernel_spmd

    nc, nch = _get_program(t_steps)
    _, nslot, _ = _dims(t_steps)

    x = np.asarray(x, np.float32)
    w_rec, bvec = _pack_weights(
        np.asarray(w_ih0, np.float32), np.asarray(w_hh0, np.float32),
        np.asarray(b_ih0, np.float32), np.asarray(b_hh0, np.float32),
        np.asarray(w_ih1, np.float32), np.asarray(w_hh1, np.float32),
        np.asarray(b_ih1, np.float32), np.asarray(b_hh1, np.float32))

    in_maps = []
    for c in range(NCORES):
        xc = x[c * BL:(c + 1) * BL, :, 0]          # [BL, t]
        xt = np.zeros((nch * CHUNK, BL), np.float32)
        xt[:t_steps, :] = xc.T
        in_maps.append({
            "xt": xt.reshape(nch, CHUNK * BL),
            "w_rec": w_rec, "bvec": bvec,
        })

    res = run_bass_kernel_spmd(nc, in_maps, list(range(NCORES)), trace=trace)

    w_lin = np.asarray(w_lin, np.float32)
    b_lin = np.asarray(b_lin, np.float32)
    out = np.empty((B, t_steps, 1), np.float32)
    for c in range(NCORES):
        h2 = res.results[c]["h2"]                  # [nch, H, CHUNK*BL]
        # slot s holds layer-1 state after macro-step s-1, which processed
        # layer-1 timestep t' = s-2  =>  h2[b, t] = slot t+2
        arr = h2.reshape(nch, H, CHUNK, BL)
        arr = arr.transpose(3, 0, 2, 1).reshape(BL, nch * CHUNK, H)
        h2_bt = arr[:, 2:t_steps + 2, :]           # [BL, t, H]
        out[c * BL:(c + 1) * BL, :, 0] = h2_bt @ w_lin[0, :] + b_lin[0]
    return out, res


def kernel(x, w_ih0, w_hh0, b_ih0, b_hh0, w_ih1, w_hh1, b_ih1, b_hh1,
           w_lin, b_lin):
    out, _ = run(x, w_ih0, w_hh0, b_ih0, b_hh0, w_ih1, w_hh1, b_ih1, b_hh1,
                 w_lin, b_lin)
    return out



# revision 12
# speedup vs baseline: 1.3764x; 1.3764x over previous
"""Trainium2 Bass kernel for a 2-layer GRU (H=10) + linear head.

Strategy (pure data parallel, 8 cores):
  - Shard batch B=1024 -> 128 per core; replicate the tiny weights.
  - Per core, the T=2048 recurrence runs as one fused chain covering BOTH
    GRU layers (layer 1 lags layer 0 by one step, software-pipelined).
  - ONE matmul per macro-step (v1 had 4 PE ops): the next step's gate
    preacts distribute over the blend h' = uu - vvneg with
    uu = z*h, vvneg = (z-1)*n, so the matmul contracts
    rhs = [uu(20); pad(12); vvneg(20); x(1); ones(1)] (K=54, x+ones DMA-delivered) against a
    constant lhsT = [W; 0; -W; w_x; biases] (one LDWEIGHTS + one pass;
    f32r dtype avoids fp32's two half-speed passes).  Biases ride the
    ones-row; x rides its own row (DMA'd straight into the rhs tile).
  - PSUM [116, BL] per step: z@0:20, r@32:52, hn@64:84, xn@96:116 (pads
    keep every engine access quadrant-legal).  One sigmoid covers 0:52.
  - Chain per step: MM -> sigmoid(ACT) -> tt,narg(DVE) -> tanh(ACT) ->
    vvneg(DVE STT fusing (z-1)*n) -> MM.  Off-chain:
    Pool computes uu = z*h and h' = uu - vvneg (state for h2 output and
    the next uu).
  - h' states accumulate in rotating chunk tiles; h2 rows DMA'd out per
    chunk.  Final linear head (10 -> 1) runs on host.
"""

import ml_dtypes
import numpy as np

H = 10
B = 1024
T = 2048
NCORES = 8
BL = B // NCORES  # 128 batch rows per core
CHUNK = 32        # time slots per SBUF chunk tile

_PROGRAM_CACHE = {}


def _dims(t_steps):
    nstep = t_steps + 1          # macro-steps 0..t (layer1 lags by one)
    nslot = nstep + 1            # state slots
    nch = (nslot + CHUNK - 1) // CHUNK
    return nstep, nslot, nch


def _build_program(t_steps, mm_dt="bf16"):
    from contextlib import ExitStack

    import concourse.bass as bass
    import concourse.mybir as mybir

    fp32 = mybir.dt.float32
    f32r = mybir.dt.float32r
    Alu = mybir.AluOpType
    Act = mybir.ActivationFunctionType

    nstep, nslot, nch = _dims(t_steps)

    nc = bass.Bass()

    if mm_dt == "f32r":
        mdt = f32r
    elif mm_dt == "bf16":
        mdt = mybir.dt.float16
    else:
        mdt = fp32
    x_d = nc.declare_dram_parameter("xt", [nch, 2, CHUNK * BL], mdt,
                                    isOutput=False)
    w_d = nc.declare_dram_parameter("wcat", [54, 116], mdt, isOutput=False)
    h2_d = nc.declare_dram_parameter("h2", [nch, H, CHUNK * BL], fp32,
                                     isOutput=True)

    ctx = ExitStack()
    sb = lambda shape, name, dt=fp32: ctx.enter_context(
        nc.sbuf_tensor(name, shape, dt))
    ps = lambda shape, name: ctx.enter_context(
        nc.psum_tensor(name, shape, fp32))
    sem = lambda name: ctx.enter_context(nc.semaphore(name))

    wcat = sb([54, 116], "wcat_s", mdt)
    srz = sb([52, BL], "srz")    # sigmoid out: z@0:20, r@32:52
    vvp = sb([20, BL], "vvp")    # Pool: (1-z)*n at base 0
    omz = sb([20, BL], "omz")
    tt = sb([20, BL], "tt")
    # mstate: rows 0:20 uu, 20:32 zero pad, 32:52 vvneg, 52 x, 53 ones
    mstate = [sb([54, CHUNK * BL], f"mstate{i}", mdt) for i in range(3)]
    hstate = [sb([20, CHUNK * BL], f"hstate{i}") for i in range(3)]
    P = [ps([116, BL], "P0"), ps([116, BL], "P1")]
    narg = ps([20, BL], "narg")
    sn = sb([20, BL], "sn")

    sem_d = sem("sem_d")    # DVE: prologue=1; narg(k)->2k+2, vvneg(k)->2k+3
    sem_a = sem("sem_a")    # ACT: sigmoid(k)->2k+1, tanh(k)->2k+2
    sem_p = sem("sem_p")    # PE: matmul(k)->k+1
    sem_g = sem("sem_g")    # Pool: uu->4k+1, omz->4k+2, vvpos->4k+3, h'->4k+4
    dma_w = sem("dma_w")    # weight DMA
    dma_x = [sem("dma_xa"), sem("dma_xb"), sem("dma_xc")]  # x per c%3
    dma_o = [sem(f"dma_o{i}") for i in range(3)]  # h2 out, by c%3

    def mslot(s, rows):
        return mstate[(s // CHUNK) % 3][rows, (s % CHUNK) * BL:
                                        (s % CHUNK) * BL + BL]

    def hslot(s, rows=slice(0, 20)):
        return hstate[(s // CHUNK) % 3][rows, (s % CHUNK) * BL:
                                        (s % CHUNK) * BL + BL]


    with nc.Block() as block:

        @block.sync
        def _(sp):
            sp.dma_start(wcat[:, :], w_d[:]).then_inc(dma_w, 16)
            for c in range(min(3, nch)):
                sp.dma_start(mstate[c][52:54, :], x_d[c]).then_inc(
                    dma_x[c], 16)
            for c in range(nch):
                # stream out chunk c once its last h' lands
                last_k = min(CHUNK * c + CHUNK - 2, nstep - 1)
                sp.wait_ge(sem_g, 4 * last_k + 4)
                sp.dma_start(h2_d[c], hstate[c % 3][H:2 * H, :]).then_inc(
                    dma_o[c % 3], 16)
                # refill x row of tile c%3 (chunk c+3) once chunk c's MMs done
                if c + 3 < nch:
                    sp.wait_ge(sem_p, CHUNK * c + CHUNK)
                    sp.dma_start(mstate[c % 3][52:54, :],
                                 x_d[c + 3]).then_inc(dma_x[c % 3], 16)

        @block.tensor
        def _(pe):
            for k in range(nstep):
                if k % CHUNK == 0:
                    c = k // CHUNK
                    pe.wait_ge(dma_x[c % 3], 16 * (c // 3 + 1))
                    if k == 0:
                        pe.wait_ge(dma_w, 16)
                        pe.wait_ge(sem_d, 1)   # DVE prologue memsets done
                if k > 0:
                    pe.wait_ge(sem_d, 2 * k + 1)      # vvneg(k-1): 2(k-1)+3
                    pe.wait_ge(sem_g, 4 * k - 2)      # omz(k-1): srz reads done
                nc.tensor.matmul(P[k % 2][:, :], wcat[:, :],
                                 mslot(k, slice(0, 54)),
                                 start=True, stop=True).then_inc(sem_p)

        @block.scalar
        def _(act):
            for k in range(nstep):
                act.wait_ge(sem_p, k + 1)
                nc.scalar.activation(srz[:, :], P[k % 2][0:52, :],
                                     Act.Sigmoid).then_inc(sem_a)
                act.wait_ge(sem_d, 2 * k + 2)         # narg(k)
                if k > 0:
                    act.wait_ge(sem_g, 4 * k - 1)     # vvpos(k-1) read sn
                nc.scalar.activation(sn[:, :], narg[:, :],
                                     Act.Tanh).then_inc(sem_a)

        @block.vector
        def _(dve):
            dve.wait_ge(dma_w, 16)
            # zero pads (all tiles, never rewritten), ones rows, and the
            # slot-0/1 state rows read before first writes
            u32 = mybir.dt.uint32
            for i in range(3):
                nc.vector.memset(mstate[i][0:32, :].bitcast(u32), 0)
            nc.vector.memset(mstate[0][32:52, 0:2 * BL].bitcast(u32), 0)
            nc.vector.memset(hstate[0][0:20, 0:2 * BL], 0.0).then_inc(sem_d)
            for k in range(nstep):
                dve.wait_ge(sem_a, 2 * k + 1)
                nc.vector.tensor_mul(tt[:, :], P[k % 2][64:84, :],
                                     srz[32:52, :])
                nc.vector.tensor_add(narg[:, :], tt[:, :],
                                     P[k % 2][96:116, :]).then_inc(sem_d)
                dve.wait_ge(sem_a, 2 * k + 2)
                # vvneg = (z - 1) * n  -> mstate slot k+1 rows 32:52
                if k == 0:
                    nc.vector.scalar_tensor_tensor(
                        mslot(1, slice(32, 42)), srz[0:10, :], 1.0,
                        sn[0:10, :], op0=Alu.subtract,
                        op1=Alu.mult).then_inc(sem_d)
                else:
                    nc.vector.scalar_tensor_tensor(
                        mslot(k + 1, slice(32, 52)), srz[0:20, :], 1.0,
                        sn[:, :], op0=Alu.subtract,
                        op1=Alu.mult).then_inc(sem_d)

        @block.gpsimd
        def _(gp):
            # sem_g: uu->4k+1, omz->4k+2, vvpos->4k+3, h'->4k+4
            for k in range(nstep):
                gp.wait_ge(sem_a, 2 * k + 1)
                # uu = z * h(k-1) -> mstate slot k+1 rows 0:20
                if k == 0:
                    nc.gpsimd.tensor_mul(mslot(1, slice(0, 10)),
                                         srz[0:10, :],
                                         hslot(0, slice(0, 10))).then_inc(
                                             sem_g)
                else:
                    nc.gpsimd.tensor_mul(mslot(k + 1, slice(0, 20)),
                                         srz[0:20, :],
                                         hslot(k)).then_inc(sem_g)
                # omz = 1 - z
                nc.gpsimd.tensor_scalar(omz[:, :], srz[0:20, :], -1.0, 1.0,
                                        op0=Alu.mult,
                                        op1=Alu.add).then_inc(sem_g)
                gp.wait_ge(sem_a, 2 * k + 2)          # tanh(k)
                nc.gpsimd.tensor_mul(vvp[:, :], omz[:, :],
                                     sn[:, :]).then_inc(sem_g)
                if (k + 1) % CHUNK == 0 and (k + 1) // CHUNK >= 3:
                    cp = (k + 1) // CHUNK
                    gp.wait_ge(dma_o[cp % 3], 16 * (cp // 3))
                # h'(k) = uu + (1-z)*n -> hstate slot k+1
                if k == 0:
                    nc.gpsimd.tensor_add(hslot(1, slice(0, 10)),
                                         mslot(1, slice(0, 10)),
                                         vvp[0:10, :]).then_inc(sem_g)
                else:
                    nc.gpsimd.tensor_add(hslot(k + 1),
                                         mslot(k + 1, slice(0, 20)),
                                         vvp[:, :]).then_inc(sem_g)

    return nc, nch


def _get_program(t_steps):
    if t_steps not in _PROGRAM_CACHE:
        _PROGRAM_CACHE[t_steps] = _build_program(t_steps)
    return _PROGRAM_CACHE[t_steps]


def _pack_weights(w_ih0, w_hh0, b_ih0, b_hh0, w_ih1, w_hh1, b_ih1, b_hh1):
    """lhsT [54, 116]: rows 0:20 contract uu=[h0;h1], rows 20:32 zero,
    rows 32:52 contract vvneg (negated), row 52 x, row 53 biases.
    cols: z0@0:10 z1@10:20 r0@32:42 r1@42:52 hn0@64:74 hn1@74:84
          xn0@96:106 xn1@106:116."""
    W = np.zeros((20, 116), np.float32)
    # rows 0:10 contract h0
    W[0:10, 0:10] = w_hh0[10:20, :].T       # z0
    W[0:10, 32:42] = w_hh0[0:10, :].T       # r0
    W[0:10, 64:74] = w_hh0[20:30, :].T      # hn0
    W[0:10, 10:20] = w_ih1[10:20, :].T      # z1 (layer1 input side)
    W[0:10, 42:52] = w_ih1[0:10, :].T       # r1
    W[0:10, 106:116] = w_ih1[20:30, :].T    # xn1
    # rows 10:20 contract h1
    W[10:20, 10:20] = w_hh1[10:20, :].T     # z1
    W[10:20, 42:52] = w_hh1[0:10, :].T      # r1
    W[10:20, 74:84] = w_hh1[20:30, :].T     # hn1

    wcat = np.zeros((54, 116), np.float32)
    wcat[0:20] = W
    wcat[32:52] = -W
    wcat[52, 0:10] = w_ih0[10:20, 0]        # z0
    wcat[52, 32:42] = w_ih0[0:10, 0]        # r0
    wcat[52, 96:106] = w_ih0[20:30, 0]      # xn0
    wcat[53, 0:10] = b_ih0[10:20] + b_hh0[10:20]    # z0
    wcat[53, 10:20] = b_ih1[10:20] + b_hh1[10:20]   # z1
    wcat[53, 32:42] = b_ih0[0:10] + b_hh0[0:10]     # r0
    wcat[53, 42:52] = b_ih1[0:10] + b_hh1[0:10]     # r1
    wcat[53, 64:74] = b_hh0[20:30]                  # hn0
    wcat[53, 74:84] = b_hh1[20:30]                  # hn1
    wcat[53, 96:106] = b_ih0[20:30]                 # xn0
    wcat[53, 106:116] = b_ih1[20:30]                # xn1
    return wcat


def run(x, w_ih0, w_hh0, b_ih0, b_hh0, w_ih1, w_hh1, b_ih1, b_hh1,
        w_lin, b_lin, t_steps=T, trace=False):
    from concourse.bass_utils import run_bass_kernel_spmd

    nc, nch = _get_program(t_steps)

    x = np.asarray(x, np.float32)
    wcat = _pack_weights(
        np.asarray(w_ih0, np.float32), np.asarray(w_hh0, np.float32),
        np.asarray(b_ih0, np.float32), np.asarray(b_hh0, np.float32),
        np.asarray(w_ih1, np.float32), np.asarray(w_hh1, np.float32),
        np.asarray(b_ih1, np.float32), np.asarray(b_hh1, np.float32))

    in_maps = []
    for c in range(NCORES):
        xc = x[c * BL:(c + 1) * BL, :, 0]          # [BL, t]
        xt = np.zeros((nch, 2, CHUNK * BL), np.float32)
        xf = np.zeros((nch * CHUNK, BL), np.float32)
        xf[:t_steps, :] = xc.T
        xt[:, 0, :] = xf.reshape(nch, CHUNK * BL)
        xt[:, 1, :] = 1.0
        in_maps.append({
            "xt": xt.astype(np.float16),
            "wcat": wcat.astype(np.float16),
        })

    res = run_bass_kernel_spmd(nc, in_maps, list(range(NCORES)), trace=trace)

    w_lin = np.asarray(w_lin, np.float32)
    b_lin = np.asarray(b_lin, np.float32)
    out = np.empty((B, t_steps, 1), np.float32)
    for c in range(NCORES):
        h2 = res.results[c]["h2"]                  # [nch, H, CHUNK*BL]
        # slot s holds layer-1 state after macro-step s-1, which processed
        # layer-1 timestep t' = s-2  =>  h2[b, t] = slot t+2
        arr = h2.reshape(nch, H, CHUNK, BL)
        arr = arr.transpose(3, 0, 2, 1).reshape(BL, nch * CHUNK, H)
        h2_bt = arr[:, 2:t_steps + 2, :]           # [BL, t, H]
        out[c * BL:(c + 1) * BL, :, 0] = h2_bt @ w_lin[0, :] + b_lin[0]
    return out, res


def kernel(x, w_ih0, w_hh0, b_ih0, b_hh0, w_ih1, w_hh1, b_ih1, b_hh1,
           w_lin, b_lin):
    out, _ = run(x, w_ih0, w_hh0, b_ih0, b_hh0, w_ih1, w_hh1, b_ih1, b_hh1,
                 w_lin, b_lin)
    return out
